# revision 18
# baseline (speedup 1.0000x reference)
"""Trainium2 Bass kernel for nn_CVX_Reasoning_Engine.

MLP (16384x512 -> 512 -> 256 -> 128 -> 64 -> 256) with LeakyReLU(0.2),
followed by a closed-form per-object/axis QP solve.

Strategy (v2):
- Pure data parallel over 8 NeuronCores (2048 batch rows each).
- fp16 everywhere off-chip and between layers (PSUM accumulates fp32):
  halves HBM traffic vs fp32 and unlocks DVE 2x/4x perf modes.
- Host-side prep: fold `bounds` into the layer-1 bias; transpose z so
  activations flow feature-major; pack small weights into one tile.
- L5 weight matrix is EXTENDED with a third column group V computing
  v = 0.5*(pp - pg + hi) directly on the PE (it is linear in h4), so
  the QP needs no scale ops. Column order per group interleaves x/y
  pairs so QP outputs write the (obj, coord) interleaved layout with
  unit-stride innermost APs (DVE fast modes).
- QP per staging (256 batch rows), lo/hi baked as immediates:
    g1 = Relu(PG - 1)              # Act, fused into PSUM->SBUF move
    a, v = copy [PP | V]           # DVE copy
    s  = (a max lo) + g1           # DVE stt  (= x0 + g0 - 1)
    w  = (s + 1) min hi            # DVE ts -> out
    t  = min(a, v)                 # DVE tt
    x  = (t min hi-1) max lo       # DVE ts -> out
- Chunks (512, 1024, 512): small first chunk starts the PE early
  behind the serialized DMA queue; small last chunk cuts the QP tail;
  L5 stagings interleave with next-chunk L1 to keep the PE warm.
- Warmup matmuls on a memset tile bridge the initial DMA wait so the
  PE is at full clock when real work lands.
"""

import numpy as np

BS, Z, NOBJ = 16384, 512, 64
NCORES = 8
BSC = BS // NCORES            # 2048 batch rows per core
P = 128
DEF_CHUNKS = (512, 1024, 512)

# packed-weight layout (per-partition fp16 offsets)
_W2O, _W3O, _W4O, _W5O = 0, 1024, 1280, 1344   # W5 ext is [65, 384]
_BIAS_F32 = 864               # f32 index where biases start (fp16 idx 1728)
_WKW = 1746

_cache = {}


def _build(b0, b1, b2, b3, reps=1, chunks=DEF_CHUNKS):
    import concourse.tile as tile
    from concourse import bacc, mybir

    f32 = mybir.dt.float32
    f16 = mybir.dt.float16
    AF = mybir.ActivationFunctionType
    Alu = mybir.AluOpType

    nc = bacc.Bacc("TRN2", target_bir_lowering=False, debug=False,
                   num_devices=NCORES)

    zt_d = nc.dram_tensor("zt", (Z, BSC), f16, kind="ExternalInput").ap()
    w1_d = nc.dram_tensor("w1", (512, 512), f16, kind="ExternalInput").ap()
    wk_d = nc.dram_tensor("wk", (P, _WKW), f16, kind="ExternalInput").ap()
    o_d = nc.dram_tensor("o", (BSC, 256), f16, kind="ExternalOutput").ap()

    lo_x, hi_x = float(b0), float(b2)
    lo_y, hi_y = float(b1), float(b3)
    eq = (b0 == b1 and b2 == b3)
    W0 = chunks[0]
    assert sum(chunks) == BSC and all(c % 256 == 0 for c in chunks)
    assert W0 <= 512  # first-chunk L1 m2/m3 PSUMs borrow 768-col ps5 tiles

    with tile.TileContext(nc) as tc:
        with (
            tc.tile_pool(name="wp", bufs=1) as wp,
            tc.tile_pool(name="zp", bufs=3) as zp,
            tc.tile_pool(name="hp", bufs=2) as hp,
            tc.tile_pool(name="op", bufs=2) as op,
            tc.tile_pool(name="tmp", bufs=3) as tmp,
            tc.tile_pool(name="big", bufs=4, space="PSUM") as big,
            tc.tile_pool(name="ps5", bufs=2, space="PSUM") as ps5p,
        ):
            WMAX = max(chunks)
            # ---- resident weights ----
            w1_sb = wp.tile([P, 4 * 512], f16, tag="w1")
            w1v = w1_d.rearrange("(k p) m -> p k m", p=P)
            wk_sb = wp.tile([P, _WKW], f16, tag="wk")

            def w1k(k):
                return w1_sb[:, k * 512:(k + 1) * 512]

            w2v = wk_sb[:, _W2O:_W2O + 1024]
            w3v = wk_sb[:, _W3O:_W3O + 256]
            w4v = wk_sb[:, _W4O:_W4O + 64]
            w5v = wk_sb[:, _W5O:_W5O + 384]
            bias = wk_sb[:, 2 * _BIAS_F32:2 * _BIAS_F32 + 18].bitcast(f32)
            b1v = bias[:, 0:4]
            b2v = bias[:, 4:6]
            b3v = bias[:, 6:7]
            b4v = bias[:, 7:8]
            neg1 = bias[:, 8:9]

            # ---- PE warmup on a memset tile (bridges initial DMA wait) ----
            wu_sb = tmp.tile([P, 128], f16, tag="wu")
            nc.gpsimd.memset(wu_sb[:], 0.0)
            wu_ps = ps5p.tile([P, 1024], f32, tag="l5")
            for i in range(30):
                nc.tensor.matmul(wu_ps[:, 0:128], wu_sb[:], wu_sb[:],
                                 start=True, stop=True)

            pending = []          # deferred L5/QP stagings of previous chunk
            pending_store = []    # deferred store DMA of previous chunk

            def flush_one():
                if pending:
                    pending.pop(0)()

            def flush_all():
                while pending:
                    pending.pop(0)()

            for rep in range(reps):
              col0 = 0
              for ci, W in enumerate(chunks):
                first = (rep == 0 and ci == 0)
                hfs = []
                off = 0
                while off < W:
                    hw = min(512, W - off)
                    hfs.append((off, hw))
                    off += hw

                # ---- load z chunk (feature-major) ----
                zt_n = zp.tile([P, 4 * WMAX], f16, tag="zt")
                if first:
                    # interleave w1/z k-pieces + early bias mini-DMA so the
                    # PE and Act start as soon as possible
                    bo = 2 * _BIAS_F32
                    nc.sync.dma_start(w1_sb[:, 0:512], w1v[:, 0, :])
                    nc.sync.dma_start(zt_n[:, 0:W], zt_d[0:P, 0:W])
                    nc.sync.dma_start(wk_sb[:, bo:_WKW], wk_d[:, bo:_WKW])
                    for k in range(1, 4):
                        nc.sync.dma_start(w1_sb[:, k * 512:(k + 1) * 512],
                                          w1v[:, k, :])
                        nc.sync.dma_start(zt_n[:, k * W:(k + 1) * W],
                                          zt_d[k * P:(k + 1) * P, 0:W])
                    nc.sync.dma_start(wk_sb[:, 0:bo], wk_d[:, 0:bo])
                else:
                    nc.sync.dma_start(
                        zt_n[:, 0:4 * W].rearrange("p (k c) -> p k c", k=4),
                        zt_d[:, col0:col0 + W]
                            .rearrange("(k p) c -> p k c", p=P))

                def zk(k, W=W, zt_n=zt_n):
                    return zt_n[:, k * W:(k + 1) * W]

                # ---- L1: 512 -> 512 ----
                h1_n = hp.tile([P, 4 * WMAX], f16, tag="h1")

                def l1_act(m, pst, off=0, hw=None, W=W, h1_n=h1_n):
                    hw = W if hw is None else hw
                    nc.scalar.activation(
                        h1_n[:, m * WMAX + off:m * WMAX + off + hw],
                        pst[:, 0:hw],
                        AF.Prelu, bias=b1v[:, m:m + 1], alpha=0.2)

                if first:
                    # k-outer over all four m-tiles: PE consumes each z
                    # k-piece as it lands (PSUM from big + ps5 pools)
                    ps_m0 = big.tile([P, 512], f32, tag="big")
                    ps_m1 = big.tile([P, 512], f32, tag="big")
                    ps_m2 = big.tile([P, 512], f32, tag="big")
                    ps_m3 = big.tile([P, 512], f32, tag="big")
                    ps_m = [ps_m0, ps_m1, ps_m2, ps_m3]
                    for k in range(4):
                        for m in range(4):
                            nc.tensor.matmul(
                                ps_m[m][:, 0:W],
                                w1k(k)[:, m * 128:(m + 1) * 128],
                                zk(k),
                                start=(k == 0), stop=(k == 3))
                    for m in range(4):
                        l1_act(m, ps_m[m])
                else:
                    for m in range(4):
                        for off, hw in hfs:
                            pst = big.tile([P, 512], f32, tag="big")
                            for k in range(4):
                                nc.tensor.matmul(
                                    pst[:, 0:hw],
                                    w1k(k)[:, m * 128:(m + 1) * 128],
                                    zk(k)[:, off:off + hw],
                                    start=(k == 0), stop=(k == 3))
                            l1_act(m, pst, off, hw)
                        flush_one()
                    flush_all()
                # store of the previous chunk: emitted only after that
                # chunk's QP writers (flushed above) are in program order
                while pending_store:
                    pending_store.pop(0)()

                def h1k(k, W=W, h1_n=h1_n):
                    return h1_n[:, k * WMAX:k * WMAX + W]

                # ---- L2: 512 -> 256 ----
                h2_n = hp.tile([P, 2 * WMAX], f16, tag="h2")
                for m in range(2):
                    for off, hw in hfs:
                        pst = big.tile([P, 512], f32, tag="big")
                        for k in range(4):
                            nc.tensor.matmul(
                                pst[:, 0:hw],
                                w2v[:, k * 256 + m * 128:k * 256 + (m + 1) * 128],
                                h1k(k)[:, off:off + hw],
                                start=(k == 0), stop=(k == 3))
                        nc.scalar.activation(
                            h2_n[:, m * WMAX + off:m * WMAX + off + hw],
                            pst[:, 0:hw],
                            AF.Prelu, bias=b2v[:, m:m + 1], alpha=0.2)
                    flush_one()

                # ---- L3: 256 -> 128 ----
                h3_n = hp.tile([P, WMAX], f16, tag="h3")
                for off, hw in hfs:
                    pst = big.tile([P, 512], f32, tag="big")
                    for k in range(2):
                        nc.tensor.matmul(
                            pst[:, 0:hw],
                            w3v[:, k * 128:(k + 1) * 128],
                            h2_n[:, k * WMAX + off:k * WMAX + off + hw],
                            start=(k == 0), stop=(k == 1))
                    nc.scalar.activation(h3_n[:, off:off + hw], pst[:, 0:hw],
                                         AF.Prelu, bias=b3v[:, 0:1],
                                         alpha=0.2)
                flush_one()

                # ---- L4: 128 -> 64 (plus ones row for L5 bias) ----
                h4_n = hp.tile([65, WMAX], f16, tag="h4")
                for off, hw in hfs:
                    pst = big.tile([P, 512], f32, tag="big")
                    nc.tensor.matmul(pst[0:64, 0:hw],
                                     w4v[:], h3_n[:, off:off + hw],
                                     start=True, stop=True)
                    nc.scalar.activation(h4_n[0:64, off:off + hw],
                                         pst[0:64, 0:hw],
                                         AF.Prelu, bias=b4v[0:64, 0:1],
                                         alpha=0.2)
                nc.gpsimd.memset(h4_n[64:65, 0:W], 1.0)
                flush_all()

                # ---- L5 + QP per staging of 2 subtiles (256 batch rows) ----
                # o_sb collects the whole chunk; one store DMA per chunk.
                # QP math reads PSUM directly (no staging copy); deferred
                # stagings interleave with the next chunk's L1 on the PE.
                nstg = W // 256
                o_sb = op.tile([P, 2 * WMAX], f16, tag="o")

                def l5_qp(st, W=W, h4_n=h4_n, o_sb=o_sb):
                    # one PSUM bank (512 f32) per subtile: a matmul's
                    # output must not straddle a bank boundary
                    p5 = ps5p.tile([P, 1024], f32, tag="l5")
                    for j in range(2):
                        sub = st * 2 + j
                        nc.tensor.matmul(
                            p5[:, j * 512:j * 512 + 384],
                            h4_n[0:65, sub * P:(sub + 1) * P],
                            w5v[0:65, :], start=True, stop=True)

                    def qp_ops(p5=p5, st=st, o_sb=o_sb, act_g1=False):
                        # PSUM 5D views -> [p, s, o, c]
                        p5v = p5[:].rearrange(
                            "p (s g o c) -> p s g o c", s=2, g=4, o=NOBJ)
                        PP = p5v[:, :, 0, :, :]
                        PG = p5v[:, :, 1, :, :]
                        V = p5v[:, :, 2, :, :]
                        g1 = tmp.tile([P, 256], f16, tag="g1")
                        s0 = tmp.tile([P, 256], f16, tag="s0")
                        t1 = tmp.tile([P, 256], f16, tag="t1")
                        vc = tmp.tile([P, 256], f16, tag="vc")
                        g2 = g1[:].rearrange("p (s o c) -> p s o c",
                                             s=2, o=NOBJ)
                        s2 = s0[:].rearrange("p (s o c) -> p s o c",
                                             s=2, o=NOBJ)
                        t2 = t1[:].rearrange("p (s o c) -> p s o c",
                                             s=2, o=NOBJ)
                        v2 = vc[:].rearrange("p (s o c) -> p s o c",
                                             s=2, o=NOBJ)
                        # V -> SBUF (DVE can't read 2 PSUM srcs; Pool has
                        # no PSUM port at all)
                        nc.vector.tensor_copy(v2, V)
                        ov = o_sb[:, st * 512:(st + 1) * 512].rearrange(
                            "p (s o c) -> p s o c", s=2, o=NOBJ)
                        if eq:
                            groups = [(slice(0, 2), slice(0, 2),
                                       slice(2, 4), lo_x, hi_x)]
                        else:
                            groups = [(slice(0, 1), slice(0, 1),
                                       slice(2, 3), lo_x, hi_x),
                                      (slice(1, 2), slice(1, 2),
                                       slice(3, 4), lo_y, hi_y)]
                        for cs, cx, cw, lo, hi in groups:
                            # g1 = relu(pg - 1)
                            if act_g1:
                                nc.scalar.activation(
                                    g2[:, :, :, cs], PG[:, :, :, cs],
                                    AF.Prelu, bias=neg1, alpha=0.0)
                            else:
                                nc.vector.tensor_scalar(
                                    g2[:, :, :, cs], PG[:, :, :, cs],
                                    1.0, 0.0, Alu.subtract, Alu.max)
                            # t = min(pp, v)
                            nc.vector.tensor_tensor(
                                t2[:, :, :, cs], PP[:, :, :, cs],
                                v2[:, :, :, cs], Alu.min)
                            # x = (t min hi-1) max lo
                            nc.vector.tensor_scalar(
                                ov[:, :, :, cx], t2[:, :, :, cs],
                                hi - 1.0, lo, Alu.min, Alu.max)
                            # s = (pp max lo) + g1  (= x0 + g0 - 1)
                            nc.vector.scalar_tensor_tensor(
                                s2[:, :, :, cs], PP[:, :, :, cs], lo,
                                g2[:, :, :, cs], Alu.max, Alu.add)
                            # w = (s + 1) min hi
                            nc.vector.tensor_scalar(
                                ov[:, :, :, cw], s2[:, :, :, cs],
                                1.0, hi, Alu.add, Alu.min)

                    pending.append(qp_ops)

                last = (rep == reps - 1 and ci == len(chunks) - 1)
                for st in range(nstg):
                    l5_qp(st)
                if last:
                    fin = pending.pop()
                    while pending:
                        pending.pop(0)()
                    fin(act_g1=True)

                # ---- store chunk -> DRAM (deferred past next z load) ----
                def store_chunk(col0=col0, W=W, o_sb=o_sb):
                    nc.sync.dma_start(
                        o_d[col0:col0 + W, :].rearrange(
                            "(s p) f -> p s f", p=P),
                        o_sb[:, 0:2 * W].rearrange(
                            "p (s f) -> p s f", f=256))
                if last:
                    store_chunk()
                else:
                    pending_store.append(store_chunk)
                col0 += W

    nc.compile()
    return nc


def _get_nc(b0, b1, b2, b3, reps=1, chunks=DEF_CHUNKS):
    key = (b0, b1, b2, b3, reps, tuple(chunks))
    if key not in _cache:
        _cache[key] = _build(b0, b1, b2, b3, reps, chunks)
    return _cache[key]


def _prep_inputs(z, bounds, W1, c1, W2, c2, W3, c3, W4, c4, W5, c5):
    b = np.asarray(bounds, np.float32)
    f16 = np.float16
    W1m = np.ascontiguousarray(W1[:Z], f16)
    b1 = (np.asarray(c1, np.float32)
          + b @ np.asarray(W1[Z:], np.float32)).astype(np.float32)

    wk = np.zeros((P, _WKW), f16)
    wk[:, _W2O:_W2O + 1024] = (np.asarray(W2, f16)
                               .reshape(4, P, 256).transpose(1, 0, 2)
                               .reshape(P, 1024))
    wk[:, _W3O:_W3O + 256] = (np.asarray(W3, f16)
                              .reshape(2, P, 128).transpose(1, 0, 2)
                              .reshape(P, 256))
    wk[:, _W4O:_W4O + 64] = np.asarray(W4, f16)

    # ---- extended L5 weights [65, 384] ----
    # groups: PP=[px0,py0,px1,py1,...], PG=[pw0,ph0,...], V=[vx0,vy0,...]
    # with v = 0.5*(pp - pg + hi)
    W5f = np.asarray(W5, np.float32)          # (64, 256)
    c5f = np.asarray(c5, np.float32)          # (256,)
    W5q = W5f.reshape(64, NOBJ, 4)
    c5q = c5f.reshape(NOBJ, 4)
    hi_xy = np.array([b[2], b[3]], np.float32)
    w5x = np.zeros((65, 384), np.float32)
    # PP
    w5x[:64, 0:128] = W5q[:, :, 0:2].reshape(64, 128)
    w5x[64, 0:128] = c5q[:, 0:2].reshape(128)
    # PG
    w5x[:64, 128:256] = W5q[:, :, 2:4].reshape(64, 128)
    w5x[64, 128:256] = c5q[:, 2:4].reshape(128)
    # V
    w5x[:64, 256:384] = (0.5 * (W5q[:, :, 0:2] - W5q[:, :, 2:4])).reshape(64, 128)
    w5x[64, 256:384] = (0.5 * (c5q[:, 0:2] - c5q[:, 2:4]
                               + hi_xy[None, :])).reshape(128)
    wk[0:65, _W5O:_W5O + 384] = w5x.astype(f16)

    # ---- biases, stored as f32 bit-pattern in fp16 pairs ----
    bias = np.zeros((P, 9), np.float32)
    bias[:, 0:4] = b1.reshape(4, P).T
    bias[:, 4:6] = np.asarray(c2, np.float32).reshape(2, P).T
    bias[:, 6] = np.asarray(c3, np.float32)
    bias[0:64, 7] = np.asarray(c4, np.float32)
    bias[:, 8] = -1.0
    wk[:, 2 * _BIAS_F32:2 * _BIAS_F32 + 18] = bias.view(f16)

    zT = np.ascontiguousarray(np.asarray(z, f16).T)
    common = {"w1": W1m, "wk": wk}
    in_maps = []
    for i in range(NCORES):
        m = dict(common)
        m["zt"] = np.ascontiguousarray(zT[:, i * BSC:(i + 1) * BSC])
        in_maps.append(m)
    return in_maps, (float(b[0]), float(b[1]), float(b[2]), float(b[3]))


def kernel(z, bounds, W1, c1, W2, c2, W3, c3, W4, c4, W5, c5):
    from concourse.bass_utils import run_bass_kernel_spmd

    in_maps, bvals = _prep_inputs(z, bounds, W1, c1, W2, c2, W3, c3,
                                  W4, c4, W5, c5)
    nc = _get_nc(*bvals)
    res = run_bass_kernel_spmd(nc, in_maps, core_ids=list(range(NCORES)))
    out = np.concatenate([r["o"] for r in res.results], axis=0)
    return out.reshape(BS, NOBJ, 4).astype(np.float32)
